# revision 1
# baseline (speedup 1.0000x reference)
"""Trainium2 Bass kernel for GuidedFilterHR (bilateral-weighted guided filter).

Strategy (per NeuronCore, 8 cores, column-sharded):
  - Layout: column-major tiles [partition = image column, free = image row].
    Core k owns columns [128k, 128k+128); all 512 rows (no row halo needed).
  - 5x5 zero-padded box filter: vertical pass on DVE/GPSIMD (free-dim shifts),
    horizontal pass on TensorE as Toeplitz-band matmuls (incl. column halo).
  - Bilateral weighted sums: for each window offset (dy,dx), the range weight
    w = exp(-c*d^2 + ln(spatial)) is computed on ScalarE (Square + Exp with
    per-partition bias that also encodes image-border masking). Products on
    DVE/GPSIMD (bf16), and all 7 running sums (den_B, M1=sum w*dX, M2=sum w*dX^2,
    N1=sum w*dY, C=sum w*dX*dY, den_D, num_D) accumulate in fp32 PSUM via
    TensorE identity matmuls (start/stop accumulation), keeping VectorE to
    ~4 ops per offset.
  - Centered-moment identities make the guided-filter output:
      out = yb + N1/den - A*(M1/den) + Xdet + num_D/den_D,
      A = (C/den - (M1/den)(N1/den)) / (M2/den - (M1/den)^2 + 1e-6)
    which is algebraically identical to the reference (verified to 2.5e-5 in
    fp32, 2.8e-3 with the bf16 pipeline).
  - The reference's width chunking (CHUNK=64, OVL=10, TRIM=5) is mathematically
    equivalent to full-width processing because kept pixels' windows never
    cross chunk borders; borders of the *image* are handled by masking via the
    exp bias (-50 => w=0) and by restricting accumulation row ranges.
"""

import math
import numpy as np

# ---------------------------------------------------------------------------
# Problem constants (hardcoded; kernel.py must be self-contained)
# ---------------------------------------------------------------------------
M, N = 512, 1024          # image rows, cols
NCORES = 8
CW = N // NCORES          # 128 columns per core
HW_ = 8                   # halo width stored each side (box needs 7; 8 for pad)
RB = 5                    # bilateral B radius (11x11)
RD = 2                    # bilateral D radius (5x5)
DEN_B = (121 / 4.0) ** 2  # spatial denom for 11x11
DEN_D = (25 / 4.0) ** 2   # spatial denom for 5x5
MASK_BIAS = -50.0

_PROGRAM_CACHE = {}


def _build_program():
    import concourse.bacc as bacc
    import concourse.tile as tile
    import concourse.mybir as mybir
    from concourse import bass

    f32 = mybir.dt.float32
    bf16 = mybir.dt.bfloat16
    Alu = mybir.AluOpType
    Act = mybir.ActivationFunctionType

    nc = bacc.Bacc("TRN2", target_bir_lowering=False, debug=False,
                   num_devices=NCORES)

    # ---------------- DRAM I/O ----------------
    d_xm = nc.dram_tensor("xm", [CW, M], f32, kind="ExternalInput").ap()
    d_xh = nc.dram_tensor("xh", [2 * HW_, M], f32, kind="ExternalInput").ap()
    d_ym = nc.dram_tensor("ym", [CW, M], f32, kind="ExternalInput").ap()
    d_yh = nc.dram_tensor("yh", [2 * HW_, M], f32, kind="ExternalInput").ap()
    d_biasB = nc.dram_tensor("biasB", [CW, 121], f32, kind="ExternalInput").ap()
    d_biasD = nc.dram_tensor("biasD", [CW, 25], f32, kind="ExternalInput").ap()
    d_sqc = nc.dram_tensor("sqc", [CW, 1], f32, kind="ExternalInput").ap()
    d_ident = nc.dram_tensor("ident", [CW, CW], bf16, kind="ExternalInput").ap()
    d_tmm = nc.dram_tensor("tmm", [CW, CW], f32, kind="ExternalInput").ap()
    d_thm = nc.dram_tensor("thm", [2 * HW_, CW], f32, kind="ExternalInput").ap()
    d_tmh = nc.dram_tensor("tmh", [CW, 2 * HW_], f32, kind="ExternalInput").ap()
    d_thh = nc.dram_tensor("thh", [2 * HW_, 2 * HW_], f32, kind="ExternalInput").ap()
    d_out = nc.dram_tensor("outT", [CW, M], f32, kind="ExternalOutput").ap()

    with tile.TileContext(nc) as tc:
        with tc.tile_pool(name="cst", bufs=1) as cst, \
             tc.tile_pool(name="per", bufs=1) as per, \
             tc.tile_pool(name="wrk", bufs=4) as wrk, \
             tc.tile_pool(name="ps", bufs=1, space="PSUM") as ps:

            # ---------------- load constants + inputs ----------------
            xm = cst.tile([CW, M], f32, name="xm_s", tag="xm_s")
            xh = cst.tile([2 * HW_, M], f32, name="xh_s", tag="xh_s")
            ym = cst.tile([CW, M], f32, name="ym_s", tag="ym_s")
            yh = cst.tile([2 * HW_, M], f32, name="yh_s", tag="yh_s")
            biasB = cst.tile([CW, 121], f32, name="biasB_s", tag="biasB_s")
            biasD = cst.tile([CW, 25], f32, name="biasD_s", tag="biasD_s")
            sqc = cst.tile([CW, 1], f32, name="sqc_s", tag="sqc_s")
            ident = cst.tile([CW, CW], bf16, name="ident_s", tag="ident_s")
            tmm = cst.tile([CW, CW], f32, name="tmm_s", tag="tmm_s")
            thm = cst.tile([2 * HW_, CW], f32, name="thm_s", tag="thm_s")
            tmh = cst.tile([CW, 2 * HW_], f32, name="tmh_s", tag="tmh_s")
            thh = cst.tile([2 * HW_, 2 * HW_], f32, name="thh_s", tag="thh_s")
            for dst, src in [(xm, d_xm), (xh, d_xh), (ym, d_ym), (yh, d_yh),
                             (biasB, d_biasB), (biasD, d_biasD), (sqc, d_sqc),
                             (ident, d_ident), (tmm, d_tmm), (thm, d_thm),
                             (tmh, d_tmh), (thh, d_thh)]:
                nc.sync.dma_start(dst[:], src[:])

            # ---------------- 5x5 box: vertical pass (zero-padded) ----------
            vxm = per.tile([CW, M], f32, name="vxm", tag="vxm")
            vxh = per.tile([2 * HW_, M], f32, name="vxh", tag="vxh")
            vym = per.tile([CW, M], f32, name="vym", tag="vym")
            vyh = per.tile([2 * HW_, M], f32, name="vyh", tag="vyh")

            def vbox(eng, dst, src):
                eng.tensor_copy(dst[:], src[:])
                for dy in (-2, -1, 1, 2):
                    lo, hi = max(0, -dy), M - max(0, dy)
                    eng.tensor_tensor(dst[:, lo:hi], dst[:, lo:hi],
                                      src[:, lo + dy:hi + dy], Alu.add)

            vbox(nc.vector, vxm, xm)
            vbox(nc.vector, vxh, xh)
            vbox(nc.gpsimd, vym, ym)
            vbox(nc.gpsimd, vyh, yh)

            # ---------------- 5x5 box: horizontal pass on PE ----------------
            # (Toeplitz band matrices carry the 1/25 factor)
            psXb = ps.tile([CW, M], f32, tag="a0", name="psXb")
            psXbh = ps.tile([2 * HW_, M], f32, tag="a1", name="psXbh")
            psYb = ps.tile([CW, M], f32, tag="a2", name="psYb")
            psYbh = ps.tile([2 * HW_, M], f32, tag="a3", name="psYbh")
            for (pm, ph, vm, vh) in [(psXb, psXbh, vxm, vxh),
                                     (psYb, psYbh, vym, vyh)]:
                nc.tensor.matmul(pm[:], tmm[:], vm[:], start=True, stop=False)
                nc.tensor.matmul(pm[:], thm[:], vh[:], start=False, stop=True)
                nc.tensor.matmul(ph[:], tmh[:], vm[:], start=True, stop=False)
                nc.tensor.matmul(ph[:], thh[:], vh[:], start=False, stop=True)

            # ---------------- evacuate box results ----------------
            Xb_f = per.tile([CW, M], f32, name="Xb_f", tag="Xb_f")
            yb_f = per.tile([CW, M], f32, name="yb_f", tag="yb_f")
            Xbh_f = per.tile([2 * HW_, M], f32, name="Xbh_f", tag="Xbh_f")
            ybh_f = per.tile([2 * HW_, M], f32, name="ybh_f", tag="ybh_f")
            Xb_b = per.tile([CW, M], bf16, name="Xb_b", tag="Xb_b")
            yb_b = per.tile([CW, M], bf16, name="yb_b", tag="yb_b")
            nc.scalar.copy(Xb_f[:], psXb[:])
            nc.scalar.copy(yb_f[:], psYb[:])
            nc.scalar.copy(Xbh_f[:], psXbh[:])
            nc.scalar.copy(ybh_f[:], psYbh[:])
            nc.scalar.copy(Xb_b[:], psXb[:])
            nc.scalar.copy(yb_b[:], psYb[:])

            # ---------------- detail tensors ----------------
            xd_f = per.tile([CW, M], f32, name="xd_f", tag="xd_f")
            yd_f = per.tile([CW, M], f32, name="yd_f", tag="yd_f")
            xd_b = per.tile([CW, M], bf16, name="xd_b", tag="xd_b")
            z_b = per.tile([CW, M], bf16, name="z_b", tag="z_b")
            xdh_f = per.tile([2 * HW_, M], f32, name="xdh_f", tag="xdh_f")
            ydh_f = per.tile([2 * HW_, M], f32, name="ydh_f", tag="ydh_f")
            xdh_b = per.tile([2 * HW_, M], bf16, name="xdh_b", tag="xdh_b")
            zh_b = per.tile([2 * HW_, M], bf16, name="zh_b", tag="zh_b")
            nc.vector.tensor_tensor(xd_f[:], xm[:], Xb_f[:], Alu.subtract)
            nc.gpsimd.tensor_tensor(yd_f[:], ym[:], yb_f[:], Alu.subtract)
            nc.vector.tensor_tensor(z_b[:], yd_f[:], xd_f[:], Alu.subtract)
            nc.vector.tensor_copy(xd_b[:], xd_f[:])
            nc.gpsimd.tensor_tensor(xdh_f[:], xh[:], Xbh_f[:], Alu.subtract)
            nc.gpsimd.tensor_tensor(ydh_f[:], yh[:], ybh_f[:], Alu.subtract)
            nc.gpsimd.tensor_tensor(zh_b[:], ydh_f[:], xdh_f[:], Alu.subtract)
            nc.gpsimd.tensor_copy(xdh_b[:], xdh_f[:])

            # ---------------- horizontal-shift materializations -------------
            # dst[p] = src(col c0+p+dx); cross-partition moves must go via DMA
            # (compute engines require 32-aligned start partitions).
            def hshift(eng, dst, src_main, src_halo, dx):
                if dx > 0:
                    nc.sync.dma_start(dst[0:CW - dx, :], src_main[dx:CW, :])
                    nc.sync.dma_start(dst[CW - dx:CW, :],
                                      src_halo[HW_:HW_ + dx, :])
                else:
                    nc.sync.dma_start(dst[-dx:CW, :], src_main[0:CW + dx, :])
                    nc.sync.dma_start(dst[0:-dx, :],
                                      src_halo[HW_ + dx:HW_, :])

            XBs, YBs = {}, {}
            for dx in range(-RB, RB + 1):
                if dx == 0:
                    XBs[0], YBs[0] = Xb_f, yb_f
                    continue
                sx = per.tile([CW, M], f32, name=f"xbs_{dx + RB}")
                sy = per.tile([CW, M], f32, name=f"ybs_{dx + RB}")
                # f32 shifted copies (weights use f32 differences for accuracy)
                hshift(nc.vector, sx, Xb_f, Xbh_f, dx)
                hshift(nc.gpsimd, sy, yb_f, ybh_f, dx)
                XBs[dx], YBs[dx] = sx, sy
            XDs, ZSs = {}, {}
            for dx in range(-RD, RD + 1):
                if dx == 0:
                    XDs[0], ZSs[0] = xd_b, z_b
                    continue
                sx = per.tile([CW, M], bf16, name=f"xds_{dx + RD}")
                sz = per.tile([CW, M], bf16, name=f"zs_{dx + RD}")
                hshift(nc.vector, sx, xd_b, xdh_b, dx)
                hshift(nc.gpsimd, sz, z_b, zh_b, dx)
                XDs[dx], ZSs[dx] = sx, sz

            # ---------------- PSUM accumulators ----------------
            # (reuse the 4 box bank tags + 3 more; 7 of 8 banks)
            den = ps.tile([CW, M], f32, tag="a0", name="acc_den")
            M1 = ps.tile([CW, M], f32, tag="a1", name="acc_m1")
            M2 = ps.tile([CW, M], f32, tag="a2", name="acc_m2")
            N1 = ps.tile([CW, M], f32, tag="a3", name="acc_n1")
            CC = ps.tile([CW, M], f32, tag="a4", name="acc_c")
            denD = ps.tile([CW, M], f32, tag="a5", name="acc_dend")
            numD = ps.tile([CW, M], f32, tag="a6", name="acc_numd")

            # ---------------- wB: 11x11 bilateral on Xbase ----------------
            offsB = [(dy, dx) for dx in range(-RB, RB + 1)
                     for dy in range(-RB, RB + 1)]
            # first/last offsets must span the full row range (full-bank
            # start=True zeroing / stop=True group close): put dy=0 at ends.
            offsB.remove((0, -RB)); offsB.remove((0, RB))
            offsB = [(0, -RB)] + offsB + [(0, RB)]
            nB = len(offsB)
            for i, (dy, dx) in enumerate(offsB):
                t = (dy + RB) * 11 + (dx + RB)
                lo, hi = max(0, -dy), M - max(0, dy)
                L = hi - lo
                st, sp = (i == 0), (i == nB - 1)
                XS = XBs[dx][:, lo + dy:hi + dy]
                YS = YBs[dx][:, lo + dy:hi + dy]
                d = wrk.tile([CW, L], bf16, tag="d", name=f"d_{i}")
                e = wrk.tile([CW, L], bf16, tag="e", name=f"e_{i}")
                qq = wrk.tile([CW, L], bf16, tag="q", name=f"q_{i}")
                w = wrk.tile([CW, L], bf16, tag="w", name=f"w_{i}")
                t1 = wrk.tile([CW, L], bf16, tag="t1", name=f"t1_{i}")
                t2 = wrk.tile([CW, L], bf16, tag="t2", name=f"t2_{i}")
                t3 = wrk.tile([CW, L], bf16, tag="t3", name=f"t3_{i}")
                t4 = wrk.tile([CW, L], bf16, tag="t4", name=f"t4_{i}")
                nc.vector.tensor_tensor(d[:], XS, Xb_f[:, lo:hi], Alu.subtract)
                nc.gpsimd.tensor_tensor(e[:], YS, yb_f[:, lo:hi], Alu.subtract)
                nc.scalar.activation(qq[:], d[:], Act.Square, scale=sqc[:])
                nc.scalar.activation(w[:], qq[:], Act.Exp, scale=-1.0,
                                     bias=biasB[:, t:t + 1])
                nc.vector.tensor_tensor(t1[:], w[:], d[:], Alu.mult)
                nc.vector.tensor_tensor(t2[:], t1[:], d[:], Alu.mult)
                nc.gpsimd.tensor_tensor(t3[:], w[:], e[:], Alu.mult)
                nc.vector.tensor_tensor(t4[:], t1[:], e[:], Alu.mult)
                nc.tensor.matmul(den[:, lo:hi], ident[:], w[:], start=st, stop=sp)
                nc.tensor.matmul(M1[:, lo:hi], ident[:], t1[:], start=st, stop=sp)
                nc.tensor.matmul(M2[:, lo:hi], ident[:], t2[:], start=st, stop=sp)
                nc.tensor.matmul(N1[:, lo:hi], ident[:], t3[:], start=st, stop=sp)
                nc.tensor.matmul(CC[:, lo:hi], ident[:], t4[:], start=st, stop=sp)

            # ---------------- wD: 5x5 bilateral on Xdet ----------------
            offsD = [(dy, dx) for dx in range(-RD, RD + 1)
                     for dy in range(-RD, RD + 1)]
            offsD.remove((0, -RD)); offsD.remove((0, RD))
            offsD = [(0, -RD)] + offsD + [(0, RD)]
            nD = len(offsD)
            for i, (dy, dx) in enumerate(offsD):
                t = (dy + RD) * 5 + (dx + RD)
                lo, hi = max(0, -dy), M - max(0, dy)
                L = hi - lo
                st, sp = (i == 0), (i == nD - 1)
                d = wrk.tile([CW, L], bf16, tag="dd", name=f"dd_{i}")
                qq = wrk.tile([CW, L], bf16, tag="dq", name=f"dq_{i}")
                w = wrk.tile([CW, L], bf16, tag="dw", name=f"dw_{i}")
                tz = wrk.tile([CW, L], bf16, tag="dtz", name=f"dtz_{i}")
                nc.vector.tensor_tensor(d[:], XDs[dx][:, lo + dy:hi + dy],
                                        xd_b[:, lo:hi], Alu.subtract)
                nc.scalar.activation(qq[:], d[:], Act.Square, scale=sqc[:])
                nc.scalar.activation(w[:], qq[:], Act.Exp, scale=-1.0,
                                     bias=biasD[:, t:t + 1])
                nc.vector.tensor_tensor(tz[:], w[:], ZSs[dx][:, lo + dy:hi + dy],
                                        Alu.mult)
                nc.tensor.matmul(denD[:, lo:hi], ident[:], w[:], start=st, stop=sp)
                nc.tensor.matmul(numD[:, lo:hi], ident[:], tz[:], start=st, stop=sp)

            # ---------------- final assembly (f32) ----------------
            asm = per
            rden = asm.tile([CW, M], f32, name="rden", tag="rden")
            m1 = asm.tile([CW, M], f32, name="m1", tag="m1")
            n1 = asm.tile([CW, M], f32, name="n1", tag="n1")
            m2 = asm.tile([CW, M], f32, name="m2", tag="m2")
            cc = asm.tile([CW, M], f32, name="cc", tag="cc")
            nc.vector.reciprocal(rden[:], den[:])
            nc.vector.tensor_tensor(m1[:], M1[:], rden[:], Alu.mult)
            nc.vector.tensor_tensor(n1[:], N1[:], rden[:], Alu.mult)
            nc.vector.tensor_tensor(m2[:], M2[:], rden[:], Alu.mult)
            nc.vector.tensor_tensor(cc[:], CC[:], rden[:], Alu.mult)
            mm = asm.tile([CW, M], f32, name="mm", tag="mm")
            vx = asm.tile([CW, M], f32, name="vx", tag="vx")
            mn = asm.tile([CW, M], f32, name="mn", tag="mn")
            cxy = asm.tile([CW, M], f32, name="cxy", tag="cxy")
            nc.vector.tensor_tensor(mm[:], m1[:], m1[:], Alu.mult)
            nc.vector.tensor_tensor(vx[:], m2[:], mm[:], Alu.subtract)
            nc.gpsimd.tensor_tensor(mn[:], m1[:], n1[:], Alu.mult)
            nc.vector.tensor_tensor(cxy[:], cc[:], mn[:], Alu.subtract)
            vx1 = asm.tile([CW, M], f32, name="vx1", tag="vx1")
            rvx = asm.tile([CW, M], f32, name="rvx", tag="rvx")
            A = asm.tile([CW, M], f32, name="A", tag="A")
            am1 = asm.tile([CW, M], f32, name="am1", tag="am1")
            nc.vector.tensor_scalar_add(vx1[:], vx[:], 1e-6)
            nc.vector.reciprocal(rvx[:], vx1[:])
            nc.vector.tensor_tensor(A[:], cxy[:], rvx[:], Alu.mult)
            nc.vector.tensor_tensor(am1[:], A[:], m1[:], Alu.mult)
            o1 = asm.tile([CW, M], f32, name="o1", tag="o1")
            o2 = asm.tile([CW, M], f32, name="o2", tag="o2")
            o3 = asm.tile([CW, M], f32, name="o3", tag="o3")
            nc.gpsimd.tensor_tensor(o1[:], yb_f[:], n1[:], Alu.add)
            nc.vector.tensor_tensor(o2[:], o1[:], am1[:], Alu.subtract)
            nc.gpsimd.tensor_tensor(o3[:], o2[:], xd_f[:], Alu.add)
            rdd = asm.tile([CW, M], f32, name="rdd", tag="rdd")
            bd = asm.tile([CW, M], f32, name="bd", tag="bd")
            outf = asm.tile([CW, M], f32, name="outf", tag="outf")
            nc.vector.reciprocal(rdd[:], denD[:])
            nc.vector.tensor_tensor(bd[:], numD[:], rdd[:], Alu.mult)
            nc.vector.tensor_tensor(outf[:], o3[:], bd[:], Alu.add)
            nc.sync.dma_start(d_out[:], outf[:])

    nc.compile()
    return nc


def _get_program():
    if "nc" not in _PROGRAM_CACHE:
        _PROGRAM_CACHE["nc"] = _build_program()
    return _PROGRAM_CACHE["nc"]


def prepare_in_maps(X, y, r):
    """Host-side sharding + parameter tables. Returns list of per-core dicts."""
    X = np.asarray(X, dtype=np.float32)
    y = np.asarray(y, dtype=np.float32)
    r = np.float32(np.asarray(r))
    Xi = X[0, 0]
    yi = y[0, 0]
    sigma = r * (yi.max() - yi.min())
    c = np.float32(1.0) / np.float32((sigma / np.float32(2.0)) ** 2)
    sqc_val = np.float32(math.sqrt(c))

    XT = np.ascontiguousarray(Xi.T)   # [N, M] = [col, row]
    yT = np.ascontiguousarray(yi.T)

    # padded transposed images for halo extraction
    XTp = np.zeros((N + 2 * HW_, M), np.float32)
    XTp[HW_:HW_ + N] = XT
    yTp = np.zeros((N + 2 * HW_, M), np.float32)
    yTp[HW_:HW_ + N] = yT

    ident = np.eye(CW, dtype=np.float32)

    # Toeplitz band matrices for horizontal 5x5 box (with 1/25 folded in).
    # halo partition hp: hp<HW_ -> col c0-HW_+hp ; hp>=HW_ -> col c0+CW+(hp-HW_)
    halo_rel = np.array([(-HW_ + hp) if hp < HW_ else (CW + hp - HW_)
                         for hp in range(2 * HW_)])
    tmm = np.zeros((CW, CW), np.float32)
    thm = np.zeros((2 * HW_, CW), np.float32)
    tmh = np.zeros((CW, 2 * HW_), np.float32)
    thh = np.zeros((2 * HW_, 2 * HW_), np.float32)
    for m in range(CW):
        for k in range(CW):
            if abs(k - m) <= 2:
                tmm[k, m] = 1.0 / 25.0
        for k in range(2 * HW_):
            if abs(halo_rel[k] - m) <= 2:
                thm[k, m] = 1.0 / 25.0
    for hp in range(2 * HW_):
        mcol = halo_rel[hp]
        for k in range(CW):
            if abs(k - mcol) <= 2:
                tmh[k, hp] = 1.0 / 25.0
        for k in range(2 * HW_):
            if abs(halo_rel[k] - mcol) <= 2:
                thh[k, hp] = 1.0 / 25.0

    in_maps = []
    for core in range(NCORES):
        c0 = core * CW
        xm = XTp[HW_ + c0:HW_ + c0 + CW]
        ym_ = yTp[HW_ + c0:HW_ + c0 + CW]
        xh = np.concatenate([XTp[c0:c0 + HW_],
                             XTp[HW_ + c0 + CW:2 * HW_ + c0 + CW]], axis=0)
        yh = np.concatenate([yTp[c0:c0 + HW_],
                             yTp[HW_ + c0 + CW:2 * HW_ + c0 + CW]], axis=0)

        cols = c0 + np.arange(CW)
        biasB = np.zeros((CW, 121), np.float32)
        for dy in range(-RB, RB + 1):
            for dx in range(-RB, RB + 1):
                t = (dy + RB) * 11 + (dx + RB)
                sp = -(dy * dy + dx * dx) / DEN_B
                valid = (cols + dx >= 0) & (cols + dx < N)
                biasB[:, t] = np.where(valid, sp, MASK_BIAS)
        biasD = np.zeros((CW, 25), np.float32)
        for dy in range(-RD, RD + 1):
            for dx in range(-RD, RD + 1):
                t = (dy + RD) * 5 + (dx + RD)
                sp = -(dy * dy + dx * dx) / DEN_D
                valid = (cols + dx >= 0) & (cols + dx < N)
                biasD[:, t] = np.where(valid, sp, MASK_BIAS)

        in_maps.append({
            "xm": np.ascontiguousarray(xm),
            "xh": np.ascontiguousarray(xh),
            "ym": np.ascontiguousarray(ym_),
            "yh": np.ascontiguousarray(yh),
            "biasB": biasB,
            "biasD": biasD,
            "sqc": np.full((CW, 1), sqc_val, np.float32),
            "ident": ident,  # cast to bf16 at transfer by caller if needed
            "tmm": tmm, "thm": thm, "tmh": tmh, "thh": thh,
        })
    return in_maps


def _cast_in_maps(in_maps):
    out = []
    for m in in_maps:
        m = dict(m)
        import ml_dtypes
        m["ident"] = m["ident"].astype(ml_dtypes.bfloat16)
        out.append(m)
    return out


def gather_output(results):
    """results: list (per core) of dicts with 'outT' [CW, M]."""
    outT = np.concatenate([np.asarray(res["outT"]) for res in results], axis=0)
    return np.ascontiguousarray(outT.T)[None, None].astype(np.float32)


def kernel(X, y, r):
    from concourse import bass_utils
    nc = _get_program()
    in_maps = _cast_in_maps(prepare_in_maps(X, y, r))
    res = bass_utils.run_bass_kernel_spmd(nc, in_maps,
                                          core_ids=list(range(NCORES)))
    return gather_output(res.results)

